# revision 16
# baseline (speedup 1.0000x reference)
"""Trainium2 Bass kernel for nn_AttentionHead (B=4, S=4096, D=512).

reference:
    K = x @ Wk.T; Q = x @ Wq.T; V = x @ Wv.T            # [B,S,D]
    scores[b,s,t] = <K[b,s], Q[b,t]> / sqrt(D)
    scores[b,:,t] = -1e12 where mask[b,t]==0
    out = softmax(scores, axis=t) @ V                    # [B,S,D]

Sharding: 8 cores = 4 batches x 2 sequence halves (rows s of the score
matrix). No collectives (2-core collective measured ~40GB/s -- slower
than recomputing the projections).

Two algorithmic cuts vs the naive dataflow:

1. Key compaction: masked keys contribute EXACTLY zero (the reference's
   -1e12 fill underflows to 0 through exp), so the host gathers only the
   ~50% unmasked key columns (pure indexing, no host FLOPs).  The score
   matmul, the P@V matmul and the V projection all shrink by ~2x.  The
   kept set is padded to TK (multiple of 128, derived from the actual
   mask at build time); pad positions carry mask=0 so their exp bias
   (-1e9) zeroes them exactly like the reference.

2. Projection fusion: scores = K Q^T = x (Wk^T Wq) x^T.  With
   A := Wk^T Wq (one 512^3 matmul, 16 PE instructions) and G := x_half A,
   scores = G @ x_kept^T -- the Q projection (128 PE instructions)
   disappears; x itself is the stationary operand.

All matmuls in float32r (full PE rate at 512-wide moving operands; fp8
DoubleRow was measured at only 2x per-pass on HW, which the residual
compensation needed for accuracy eats up entirely -- not worth it).

Schedule (per core), TK = padded kept-key count (2176 for the reference
inputs).  DMA priority: wk+wq (A's operands), first xk tile, wv, all of
xq, rest of xk.  PE program order: warm-up, A, G, then the attention
chunks; the V projection is INLINED tile-by-tile into the first s-chunk's
t-loop (V tile ti right before the ti+1 score group) so it paces with the
xk DMA stream instead of stalling on it -- phase 1 proper is only A+G
(~20us) and the PE never waits for the 4.25MB xk tensor:
    per s-chunk of 512, for each kept t-tile of 128:
        [sc==0 only] V[t,:] = x_k-tile.T @ Wv^T-tiles   (PSUM, 4 MMs)
        S^T[t,s]  = sum_d x_k^T-tile.T @ G^T            (PSUM, 4 MMs)
        P^T       = exp(S^T/sqrt(D) + mbias[t])         (ACT -> f32r)
        out^T[d,s]+= V-tile.T @ P^T                     (4 MMs, PSUM acc)
        den128    += P^T                                (DVE)
    epilogue: den = ones.T @ den128 (1 MM), fast reciprocal, broadcast
    via a rank-1 matmul, out^T *= 1/den, DMA out^T.

Masking: only PAD positions are masked; mbias[t] = (padmask[t]-1)*1e9
inside the EXP zeroes them exactly.

Host passes x^T / W layouts (pure permutations/gathers; all FLOPs stay on
device).  The f32r DRAM declaration lets raw fp32 bits feed f32r matmuls
directly (baseline-verified bit path; end-to-end err ~5.5e-4).

Measured: ~195.8us HW exec (was 353.9us), ~712 PE matmuls/core at
~227ns steady; PE >94% busy inside its window; the rest is the fixed
~7.5us sequencer start + ~14us end-of-NEFF drain.  (Note: the device
clock wanders between runs -- some runs measure ~272ns/matmul uniformly,
i.e. ~+18% wall; take best-of-N.)
"""

import numpy as np

import concourse.bacc as bacc
import concourse.mybir as mybir
from concourse.bass_utils import run_bass_kernel_spmd
from concourse.tile import TileContext

B, S, D = 4, 4096, 512
SH = S // 2          # per-core s rows (half sequence)
P = 128              # partition tile
CH = 512             # free-dim chunk
KD = D // P          # 4 contraction tiles over d
SCALE = 1.0 / float(np.sqrt(D))

F32 = mybir.dt.float32
F32R = mybir.dt.float32r
COPY = mybir.ActivationFunctionType.Copy
EXP = mybir.ActivationFunctionType.Exp

_CACHE = {}


def _build(TK):
    NTK = TK // P        # kept-key tiles
    nc = bacc.Bacc(num_devices=8)
    # all inputs host-reshaped to [P, KD, *] so each tensor lands in 1-4
    # DMA instructions -- DMA-issue instructions cost ~600ns of issuing-
    # engine queue time each, and a jammed queue stalls the PE's psum
    # copies behind them
    xkT = nc.declare_dram_parameter("xkT", [P, KD, TK], F32R, isOutput=False)
    xqT = nc.declare_dram_parameter("xqT", [P, KD, SH], F32R, isOutput=False)
    wkN = nc.declare_dram_parameter("wkN", [P, KD, D], F32R, isOutput=False)
    wqN = nc.declare_dram_parameter("wqN", [P, KD, D], F32R, isOutput=False)
    wvT = nc.declare_dram_parameter("wvT", [P, KD, D], F32R, isOutput=False)
    maskT = nc.declare_dram_parameter("maskT", [P, NTK], F32, isOutput=False)
    outT = nc.declare_dram_parameter("outT", [D, SH], F32, isOutput=True)

    engs = None

    with TileContext(nc) as tc:
        with tc.tile_pool(name="pers", bufs=1) as pers:
            xk = pers.tile([P, KD, TK], F32R)     # x^T kept keys (d-tiled)
            gT = pers.tile([P, KD, SH], F32R)     # G^T local half
            vA = pers.tile([P, NTK, D], F32R)     # V kept keys (t-tiled)
            wv = pers.tile([P, KD, D], F32R)      # Wv^T (V inlined in sc=0)
            mk = pers.tile([P, NTK], F32)
            mbias = pers.tile([P, NTK], F32)
            ones = pers.tile([1, P], F32R)
            ones32 = pers.tile([1, P], F32)
            onec = pers.tile([P, 1], F32R)
            onec32 = pers.tile([P, 1], F32)

            # ---------------- phase 1: A, G ----------------
            with tc.tile_pool(name="stage", bufs=1) as stage, \
                 tc.tile_pool(name="ppsum", bufs=2, space="PSUM") as ppsum:
                wk = stage.tile([P, KD, D], F32R, tag="wk")
                wq = stage.tile([P, KD, D], F32R, tag="wq")
                xq = stage.tile([P, KD, SH], F32R, tag="xq")
                aSb = stage.tile([P, KD, D], F32R, tag="aSb")

                # PE warm-up while the first DMAs land (keeps the HAM
                # clock-gate from dropping the PE to half rate)
                warm32 = stage.tile([P, CH], F32, tag="warm32")
                warm = stage.tile([P, CH], F32R, tag="warm")
                nc.vector.memset(warm32, 0.0)
                nc.vector.tensor_copy(out=warm, in_=warm32)
                for r in range(20):
                    wps = ppsum.tile([P, CH], F32, tag="warm", bufs=2,
                                     name="wps")
                    nc.tensor.matmul(wps, warm[:, 0:P], warm,
                                     start=True, stop=True)

                # DMA issue on sync/gpsimd ONLY -- scalar must stay free
                # for the psum->SBUF copies the PE pipeline depends on
                engs = [nc.sync, nc.gpsimd]
                # priority: A's weights (split so both queues carry half),
                # then xq (G is next on the PE), then the V-path tensors
                nc.sync.dma_start(out=wk[:, 0:2, :], in_=wkN[:, 0:2, :])
                nc.gpsimd.dma_start(out=wk[:, 2:4, :], in_=wkN[:, 2:4, :])
                nc.sync.dma_start(out=wq[:, 0:2, :], in_=wqN[:, 0:2, :])
                nc.gpsimd.dma_start(out=wq[:, 2:4, :], in_=wqN[:, 2:4, :])
                # interleave xq (G's operand) with xk chunks (V/score
                # stationaries) so neither stream starves the PE
                kb = [0, 2 * P]
                while kb[-1] < TK:
                    kb.append(min(kb[-1] + CH, TK))
                nc.sync.dma_start(out=wv, in_=wvT[:, :, :])
                nq, nk = 0, 0
                for step in range(SH // CH + len(kb) - 1):
                    if step % 2 == 0 and nq < SH // CH:
                        engs[nq % 2].dma_start(
                            out=xq[:, :, nq * CH:(nq + 1) * CH],
                            in_=xqT[:, :, nq * CH:(nq + 1) * CH])
                        nq += 1
                    elif nk < len(kb) - 1:
                        lo, hi = kb[nk], kb[nk + 1]
                        engs[(nk + 1) % 2].dma_start(
                            out=xk[:, :, lo:hi], in_=xkT[:, :, lo:hi])
                        nk += 1

                # constants + pad-mask bias
                nc.sync.dma_start(out=mk, in_=maskT[:, :])
                nc.vector.memset(ones32, 1.0)
                nc.vector.tensor_copy(out=ones, in_=ones32)
                nc.vector.memset(onec32, 1.0)
                nc.vector.tensor_copy(out=onec, in_=onec32)
                # mbias = (padmask-1)*1e9: 0 kept, -1e9 pad -> exp == 0
                nc.vector.tensor_scalar(mbias, mk, -1.0, 1.0e9,
                                        mybir.AluOpType.add,
                                        mybir.AluOpType.mult)

                # A = Wk^T Wq  (psum i-chunk io -> aSb[:, io, :])
                for io in range(KD):
                    pa = ppsum.tile([P, CH], F32, tag="pa", name="pa")
                    for mt in range(KD):
                        nc.tensor.matmul(
                            pa,
                            wk[:, mt, io * P:(io + 1) * P],
                            wq[:, mt, :],
                            start=(mt == 0), stop=(mt == KD - 1))
                    nc.scalar.activation(out=aSb[:, io, :], in_=pa, func=COPY)

                # two V tiles here: fills the A->G seam (G's first
                # group waits on the last aSb copy) and thins sc0's
                # inline-V load
                for ti in range(2):
                    pv01 = ppsum.tile([P, D], F32, tag="pv", name="pv01")
                    for kd in range(KD):
                        nc.tensor.matmul(
                            pv01,
                            xk[:, kd, ti * P:(ti + 1) * P],
                            wv[:, kd, :],
                            start=(kd == 0), stop=(kd == KD - 1))
                    nc.scalar.activation(out=vA[:, ti, :], in_=pv01,
                                         func=COPY)

                # G^T = A-contracted x_half^T (a single matmul cannot
                # write wider than one 512-f32 PSUM bank)
                for c in range(SH // CH):
                    for jo in range(KD):
                        pg = ppsum.tile([P, CH], F32, tag="pg", name="pg")
                        for it in range(KD):
                            nc.tensor.matmul(
                                pg,
                                aSb[:, it, jo * P:(jo + 1) * P],
                                xq[:, it, c * CH:(c + 1) * CH],
                                start=(it == 0), stop=(it == KD - 1))
                        nc.scalar.activation(
                            out=gT[:, jo, c * CH:(c + 1) * CH], in_=pg,
                            func=COPY)

            # ------------- phase 2: attention (V inlined in sc=0) -------
            with tc.tile_pool(name="att", bufs=1) as att, \
                 tc.tile_pool(name="apsum", bufs=1, space="PSUM") as apsum:

                def v_group(ti):
                    # V[t-tile ti] = x_k-tile.T @ Wv^T; psum shares the
                    # "bc" bank (den/broadcast only run after the last
                    # v_group of the chunk)
                    pv = apsum.tile([P, D], F32, tag="bc", name="pv")
                    for kd in range(KD):
                        nc.tensor.matmul(
                            pv,
                            xk[:, kd, ti * P:(ti + 1) * P],
                            wv[:, kd, :],
                            start=(kd == 0), stop=(kd == KD - 1))
                    nc.scalar.activation(out=vA[:, ti, :], in_=pv, func=COPY)

                for sc in range(SH // CH):
                    opsum = [apsum.tile([P, CH], F32, tag=f"o{d}",
                                        name=f"opsum{d}")
                             for d in range(KD)]
                    den128 = att.tile([P, CH], F32R, tag="den128")

                    def s_group(ti, sc=sc):
                        ss = apsum.tile([P, CH], F32, tag="s", bufs=3)
                        for kd in range(KD):
                            nc.tensor.matmul(
                                ss,
                                xk[:, kd, ti * P:(ti + 1) * P],
                                gT[:, kd, sc * CH:(sc + 1) * CH],
                                start=(kd == 0), stop=(kd == KD - 1))
                        return ss

                    last = (sc == SH // CH - 1)
                    ss_cur = s_group(0)
                    for ti in range(NTK):
                        if sc == 0 and ti + 2 < NTK:
                            v_group(ti + 2)
                        ss_next = s_group(ti + 1) if ti + 1 < NTK else None
                        pt = att.tile([P, CH], F32R, tag="pt", bufs=3)
                        # pad-masked softmax numerator
                        nc.scalar.activation(out=pt, in_=ss_cur, func=EXP,
                                             scale=SCALE,
                                             bias=mbias[:, ti:ti + 1])
                        if ti == NTK - 1:
                            # den colsum early -- partial den128 while
                            # ACT runs the final EXP, the final tile's
                            # pt straight into the psum -- so the
                            # reciprocal chain hides under the final PV
                            # group instead of stalling the broadcast
                            dps = apsum.tile([1, CH], F32, tag="bc",
                                             name="dps")
                            nc.tensor.matmul(dps, onec, den128,
                                             start=True, stop=False)
                            nc.tensor.matmul(dps, onec, pt,
                                             start=False, stop=True)
                        for d in range(KD):
                            nc.tensor.matmul(
                                opsum[d],
                                vA[:, ti, d * P:(d + 1) * P],
                                pt, start=(ti == 0), stop=(ti == NTK - 1))
                        if ti == 0:
                            nc.vector.tensor_copy(out=den128, in_=pt)
                        elif ti != NTK - 1:
                            nc.vector.tensor_add(den128, den128, pt)
                        ss_cur = ss_next

                    if not last:
                        # drain psum banks via DVE first so the PE can
                        # reuse them without waiting on the recip chain
                        osb = []
                        for d in range(KD):
                            ot = att.tile([P, CH], F32, tag=f"osb{d}",
                                          name=f"osb{d}")
                            nc.vector.tensor_copy(out=ot, in_=opsum[d])
                            osb.append(ot)
                    rec = att.tile([1, CH], F32, tag="rec")
                    nc.vector.reciprocal_approx_fast(out=rec, in_=dps)
                    recr = att.tile([1, CH], F32R, tag="recr")
                    nc.vector.tensor_copy(out=recr, in_=rec)
                    bps = apsum.tile([P, CH], F32, tag="bc", name="bps")
                    nc.tensor.matmul(bps, ones, recr, start=True, stop=True)
                    bsb = att.tile([P, CH], F32, tag="bsb")
                    nc.vector.tensor_copy(out=bsb, in_=bps)
                    for d in range(KD):
                        fin = att.tile([P, CH], F32, tag=f"fin{d % 2}",
                                       name=f"fin{d}", bufs=2)
                        if last:
                            # multiply straight out of PSUM; GPSIMD can't
                            # read PSUM, so d=1,3 drain via a scalar COPY
                            # and multiply on GPSIMD -- two engine chains
                            # in parallel instead of four serial DVE ops
                            if d % 2 == 0:
                                nc.vector.tensor_mul(fin, opsum[d], bsb)
                            else:
                                ot = att.tile([P, CH], F32, tag=f"osb{d}",
                                              name=f"osb{d}")
                                nc.scalar.activation(out=ot, in_=opsum[d],
                                                     func=COPY)
                                nc.gpsimd.tensor_mul(fin, ot, bsb)
                        else:
                            meng = nc.vector if d % 2 == 0 else nc.gpsimd
                            meng.tensor_mul(fin, osb[d], bsb)
                        eng = engs[d % 2]
                        eng.dma_start(
                            out=outT[d * P:(d + 1) * P, sc * CH:(sc + 1) * CH],
                            in_=fin)

    nc.compile()
    return nc


def _pkd(a):
    """[D, X] -> [P, KD, X]: partition-major d-tiling (pure permutation)."""
    return np.ascontiguousarray(
        a.reshape(KD, P, a.shape[1]).transpose(1, 0, 2))


def make_in_maps(x, mask, Wk, Wq, Wv):
    """Host-side prep: per-core input dict. Pure permutations/gathers."""
    x = np.asarray(x, dtype=np.float32)
    mask = np.asarray(mask)
    wkN = _pkd(np.asarray(Wk, dtype=np.float32))
    wqN = _pkd(np.asarray(Wq, dtype=np.float32))
    wvT = _pkd(np.asarray(Wv, dtype=np.float32).T)

    idxs = [np.flatnonzero(mask[b]) for b in range(B)]
    TK = ((max(len(i) for i in idxs) + P - 1) // P) * P
    NTK = TK // P

    in_maps = []
    for b in range(B):
        idx = idxs[b]
        xkT = np.zeros((D, TK), dtype=np.float32)
        xkT[:, :len(idx)] = x[b][idx].T
        xkT = _pkd(xkT)
        padmask = np.zeros(TK, dtype=np.float32)
        padmask[:len(idx)] = 1.0
        maskT = np.ascontiguousarray(padmask.reshape(NTK, P).T)
        xTb = x[b].T
        for h in range(2):
            in_maps.append({
                "xkT": xkT,
                "xqT": _pkd(xTb[:, h * SH:(h + 1) * SH]),
                "wkN": wkN, "wqN": wqN, "wvT": wvT,
                "maskT": maskT,
            })
    return in_maps, TK


def kernel(x, mask, Wk, Wq, Wv):
    in_maps, TK = make_in_maps(x, mask, Wk, Wq, Wv)
    if ("nc", TK) not in _CACHE:
        _CACHE[("nc", TK)] = _build(TK)
        _CACHE["nc"] = _CACHE[("nc", TK)]   # convenience handle
    nc = _CACHE[("nc", TK)]

    res = run_bass_kernel_spmd(nc, in_maps, core_ids=list(range(8)))

    out = np.empty((B, S, D), dtype=np.float32)
    for b in range(B):
        for h in range(2):
            out[b, h * SH:(h + 1) * SH, :] = res.results[2 * b + h]["outT"].T
    return out
